# revision 24
# baseline (speedup 1.0000x reference)
"""LowRankAttention Trainium2 kernel (Bass/Tile), data-parallel over 8 NeuronCores.

Math per batch b (one batch per core):
    Q = q @ Wq^T, K = k @ Wk^T, V = v @ Wv^T          (rank projections, R=256)
    A = softmax(Q K^T / sqrt(R))                       (softmax over keys j)
    out = (A @ V) @ Wo^T

Layout/engine strategy:
  - Everything bf16 on the PE (1 cycle/row at any pstate, half the DMA and
    SBUF traffic of fp32).  Host pre-transposes to qT/kT/vT [D, S], casts to
    bf16, and pre-tiles the weights into their SBUF layout (zero-FLOP layout
    prep); output stays fp32.
  - One DMA per (tensor, 512-col chunk) = [128, 8, 512] so the ~500ns
    per-DMA descriptor overhead never throttles the input stream.
  - Fully software-pipelined flat schedule so the PE never idles (idle gaps
    also reset the PE pstate ramp):
      DMA order  wq, q0, wk, k0, wv, v0, k1, v1, k2, v2, k3, v3, q1-3, wo
      PE order   Qproj0, Kproj0, then 16 fused rounds for chunk 0
                 [Kproj(sc+1) two jts ahead | at | vproj | ev], then chunks
                 1-3 as pure at/ev rounds with Qproj(ic+1)/outproj(ic-1)
                 placed at round boundaries.
  - A^T tiles [j, i] from lhsT=K^T, rhs=Q^T; exp on ScalarE (no max needed,
    |logits| < ~7); unnormalized E^T feeds EV (lhsT=V tile) -> AV^T.
  - Softmax row sums: two interleaved bf16 accumulators on VectorE (all-bf16
    SBUF operands hit the 2x DVE mode, ~330ns per add) + one ones-matmul per
    chunk (replaces 16 per-jt ones-matmuls, saving ~13us of PE).  1/sum is
    broadcast [1,512]->[128,4] via an SBUF->SBUF DMA on the Pool queue; the
    reciprocal is emitted after the previous chunk's scales so it never
    blocks the in-order VectorE queue.
  - Output projection scales alternate VectorE/ScalarE per 512-col half so
    the PSUM ring never waits on a single engine.
  - PSUM budget exactly 8 banks: at-pipeline 2, K/Q-proj 2, vproj/outproj/
    sums 2, EV accumulators 2.
"""

import numpy as np
import ml_dtypes

import concourse.bacc as bacc
import concourse.mybir as mybir
import concourse.tile as tile
from concourse import bass_utils

F32 = mybir.dt.float32
BF16 = mybir.dt.bfloat16
AF = mybir.ActivationFunctionType

DIM, RANK, B, S = 1024, 256, 8, 2048
P = 128
NC = 512                      # moving-operand / psum free chunk
DT = DIM // P                 # 8  d-tiles
RT = RANK // P                # 2  r-tiles
SC = S // NC                  # 4  s-chunks (i-chunks)
JT = S // P                   # 16 j-tiles
JPC = JT // SC                # 4  j-tiles per s-chunk
SCALE = 1.0 / np.sqrt(np.float32(RANK))
LAG = 2                       # at -> ev pipeline distance (jt steps)


def build_program(reps: int = 1):
    """Build + compile the per-core Bass program. reps>1 wraps the whole body
    in a For_i loop (used only for wall-clock timing)."""
    nc = bacc.Bacc("TRN2", target_bir_lowering=False, debug=False)

    qT = nc.dram_tensor("qT", [DIM, S], BF16, kind="ExternalInput")
    kT = nc.dram_tensor("kT", [DIM, S], BF16, kind="ExternalInput")
    vT = nc.dram_tensor("vT", [DIM, S], BF16, kind="ExternalInput")
    # weights arrive pre-tiled in SBUF layout: [128, DT*RANK] / [128, RT*DIM]
    wql = nc.dram_tensor("wql", [P, DT * RANK], BF16, kind="ExternalInput")
    wkl = nc.dram_tensor("wkl", [P, DT * RANK], BF16, kind="ExternalInput")
    wvl = nc.dram_tensor("wvl", [P, DT * RANK], BF16, kind="ExternalInput")
    wol = nc.dram_tensor("wol", [P, RT * DIM], BF16, kind="ExternalInput")
    out = nc.dram_tensor("out", [S, DIM], BF16, kind="ExternalOutput")

    with tile.TileContext(nc) as tc:
        with tc.tile_pool(name="w", bufs=1) as wpool, \
             tc.tile_pool(name="ink", bufs=4) as kpool, \
             tc.tile_pool(name="inq", bufs=4) as qpool, \
             tc.tile_pool(name="invv", bufs=4) as vpool, \
             tc.tile_pool(name="per", bufs=1) as perpool, \
             tc.tile_pool(name="et", bufs=6) as etpool, \
             tc.tile_pool(name="ac", bufs=2) as accpool, \
             tc.tile_pool(name="av", bufs=4) as avpool, \
             tc.tile_pool(name="o", bufs=3) as opool, \
             tc.tile_pool(name="sm", bufs=4) as smpool, \
             tc.tile_pool(name="ps", bufs=2, space="PSUM") as pspool, \
             tc.tile_pool(name="pskq", bufs=2, space="PSUM") as pskqpool, \
             tc.tile_pool(name="pso", bufs=2, space="PSUM") as psopool, \
             tc.tile_pool(name="psav", bufs=2, space="PSUM") as psavpool, \
             tc.tile_pool(name="dr", bufs=2, space="DRAM") as drpool:

            def body(_i=None):
                # ---- weights (one straight [128, 2048] DMA each) ----
                wq_t = wpool.tile([P, DT, RANK], BF16, tag="wq", name="wq_t")
                wk_t = wpool.tile([P, DT, RANK], BF16, tag="wk", name="wk_t")
                wv_t = wpool.tile([P, DT, RANK], BF16, tag="wv", name="wv_t")
                wo_t = wpool.tile([P, RT, DIM], BF16, tag="wo", name="wo_t")
                ones = wpool.tile([P, 1], BF16, tag="ones", name="ones")
                nc.vector.memset(ones[:], 1.0)

                # ---- persistent projected tensors ----
                QT_t = perpool.tile([P, RT, S], BF16, tag="QT", name="QT_t")   # [r_p, rt, i]
                KT_t = perpool.tile([P, RT, S], BF16, tag="KT", name="KT_t")   # [r_p, rt, j]
                V_t = perpool.tile([P, JT, RANK], BF16, tag="V", name="V_t")   # [j_p, jt, r]

                ktiles, qtiles, vtiles = {}, {}, {}

                def dma_chunk(pool, dst, src, sc, tag, ndma=1):
                    # ndma>1 splits the chunk into per-dt-group DMAs so the
                    # first consumers can start before the whole MB arrives
                    # (used for the pipeline lead-in only).
                    t = pool.tile([P, DT, NC], BF16, tag=tag, name=f"{tag}{sc}")
                    g = DT // ndma
                    for i in range(ndma):
                        nc.sync.dma_start(
                            t[:, i * g:(i + 1) * g, :],
                            src.ap()[i * g * P:(i + 1) * g * P, sc * NC:(sc + 1) * NC]
                            .rearrange("(dt p) c -> p dt c", p=P))
                    dst[sc] = t

                # DMA issue order; all compute chases this stream.
                wq_ap = wql.ap().rearrange("p (dt r) -> p dt r", dt=DT)
                nc.sync.dma_start(wq_t[:, :DT // 2], wq_ap[:, :DT // 2])
                nc.sync.dma_start(wq_t[:, DT // 2:], wq_ap[:, DT // 2:])
                dma_chunk(qpool, qtiles, qT, 0, "q", ndma=8)
                nc.sync.dma_start(wk_t[:], wkl.ap().rearrange("p (dt r) -> p dt r", dt=DT))
                dma_chunk(kpool, ktiles, kT, 0, "k", ndma=8)
                nc.sync.dma_start(wv_t[:], wvl.ap().rearrange("p (dt r) -> p dt r", dt=DT))
                dma_chunk(vpool, vtiles, vT, 0, "v", ndma=2)
                for sc in range(1, SC):
                    dma_chunk(kpool, ktiles, kT, sc, "k")
                    dma_chunk(vpool, vtiles, vT, sc, "v")
                for sc in range(1, SC):
                    dma_chunk(qpool, qtiles, qT, sc, "q")
                nc.sync.dma_start(wo_t[:], wol.ap().rearrange("p (rt d) -> p rt d", rt=RT))

                def proj_chunk(w_t, tiles, dst, sc):
                    # [D,512] chunk -> [R partitions, 512]; rt-interleaved so each
                    # input d-tile feeds 2 back-to-back matmuls.
                    ps = [pskqpool.tile([P, NC], F32, tag="pskq", name=f"ps_p{rt}")
                          for rt in range(RT)]
                    for dt in range(DT):
                        for rt in range(RT):
                            nc.tensor.matmul(ps[rt][:], w_t[:, dt, rt * P:(rt + 1) * P],
                                             tiles[sc][:, dt, :],
                                             start=(dt == 0), stop=(dt == DT - 1))
                    for rt in range(RT):
                        nc.scalar.copy(dst[:, rt, sc * NC:(sc + 1) * NC], ps[rt][:])

                def vproj(jt):
                    sc, o = jt // JPC, (jt % JPC) * P
                    ps = psopool.tile([P, NC], F32, tag="pso", name="ps_v")
                    psv = ps[:, :RANK]
                    for dt in range(DT):
                        nc.tensor.matmul(psv, vtiles[sc][:, dt, o:o + P], wv_t[:, dt, :],
                                         start=(dt == 0), stop=(dt == DT - 1))
                    nc.scalar.copy(V_t[:, jt, :], psv)

                # ---- attention chunk state ----
                class Chunk:
                    def __init__(self, ic, pe_sums=False):
                        self.ic = ic
                        self.isl = slice(ic * NC, (ic + 1) * NC)
                        self.av_ps = [psavpool.tile([P, NC], F32, tag="av",
                                                    name=f"av{rt}_{ic}")
                                      for rt in range(RT)]
                        self.pe_sums = pe_sums
                        if pe_sums:
                            # last chunk: per-jt ones-matmuls so inv is ready
                            # ~2.5us after the last ev (short tail)
                            self.sum_ps = psopool.tile([P, NC], F32, tag="pso",
                                                       name=f"ps_sum{ic}")
                        else:
                            # two interleaved bf16 accumulators on DVE (all-
                            # bf16 SBUF operands hit the 2x DVE fast mode)
                            self.acc = [accpool.tile([P, NC], BF16, tag=f"acc{par}",
                                                     name=f"acc{par}_{ic}")
                                        for par in range(2)]
                        self.ets = {}

                def at_step(ch, jt):
                    ps = pspool.tile([P, NC], F32, tag="ps", name="ps_at")
                    for rt in range(RT):
                        nc.tensor.matmul(ps[:], KT_t[:, rt, jt * P:(jt + 1) * P],
                                         QT_t[:, rt, ch.isl],
                                         start=(rt == 0), stop=(rt == RT - 1))
                    et = etpool.tile([P, NC], BF16, tag="et", name="et")
                    nc.scalar.activation(et[:], ps[:], AF.Exp, scale=float(SCALE))
                    ch.ets[jt] = et

                def ev_step(ch, jt):
                    et = ch.ets.pop(jt)
                    for rt in range(RT):
                        nc.tensor.matmul(ch.av_ps[rt][:], V_t[:, jt, rt * P:(rt + 1) * P],
                                         et[:], start=(jt == 0), stop=(jt == JT - 1))
                    # softmax-denominator accumulation
                    if ch.pe_sums:
                        nc.tensor.matmul(ch.sum_ps[:1, :], ones[:], et[:],
                                         start=(jt == 0), stop=(jt == JT - 1))
                    else:
                        acc = ch.acc[jt % 2]
                        if jt < 2:
                            nc.vector.tensor_copy(acc[:], et[:])
                        else:
                            nc.vector.tensor_add(acc[:], acc[:], et[:])

                def finish_avt(ch):
                    """AV^T out of PSUM early so the next chunk's EV can start."""
                    avt = []
                    for rt in range(RT):
                        t = avpool.tile([P, NC], BF16, tag="avt", name=f"avt{rt}_{ch.ic}")
                        nc.vector.tensor_copy(t[:], ch.av_ps[rt][:])
                        avt.append(t)
                    return avt

                def finish_sums_a(ch):
                    """combine accumulators, row sums, inv broadcast [P, 4]."""
                    if ch.pe_sums:
                        psum = ch.sum_ps
                    else:
                        nc.vector.tensor_add(ch.acc[0][:], ch.acc[0][:], ch.acc[1][:])
                        psum = psopool.tile([P, NC], F32, tag="pso", name="ps_sum")
                        nc.tensor.matmul(psum[:1, :], ones[:], ch.acc[0][:],
                                         start=True, stop=True)
                    sums_sb = smpool.tile([1, NC], F32, tag="sums_sb", name="sums_sb")
                    nc.vector.tensor_copy(sums_sb[:], psum[:1, :])
                    # broadcast [1,512] -> [128,4] via a DRAM round-trip on the
                    # Act hwdge queue (SBUF->SBUF partition-crossing DMA and the
                    # Pool swdge path both misbehave on real hardware)
                    scr = drpool.tile([1, NC], F32, tag="scr", name="scr")
                    nc.scalar.dma_start(scr[:], sums_sb[:])
                    inv = smpool.tile([P, NC // P], F32, tag="inv", name=f"inv{ch.ic}")
                    nc.scalar.dma_start(inv[:], scr[:].rearrange("o (a p) -> p (o a)", p=P))
                    return inv

                def finish_sums_b(inv):
                    # emitted after the previous chunk's scales so the DMA wait
                    # never stalls the in-order DVE queue
                    nc.vector.reciprocal(inv[:], inv[:])

                def outproj(ic, avt, inv, its):
                    for it in its:
                        i0 = ic * NC + it * P
                        ot = opool.tile([P, DIM], BF16, tag="ot", name="ot")
                        for dc in range(DIM // NC):
                            ps = psopool.tile([P, NC], F32, tag="pso", name="ps_o")
                            for rt in range(RT):
                                nc.tensor.matmul(ps[:], avt[rt][:, it * P:(it + 1) * P],
                                                 wo_t[:, rt, dc * NC:(dc + 1) * NC],
                                                 start=(rt == 0), stop=(rt == RT - 1))
                            # alternate scale engine by it-parity so consecutive
                            # it-groups release their PSUM slots independently
                            osl = ot[:, dc * NC:(dc + 1) * NC]
                            if it % 2 == 0:
                                nc.vector.tensor_scalar_mul(osl, ps[:], inv[:, it:it + 1])
                            else:
                                nc.scalar.activation(osl, ps[:], AF.Copy,
                                                     scale=inv[:, it:it + 1])
                        nc.sync.dma_start(out.ap()[i0:i0 + P, :], ot[:])

                # ---- flat schedule ----
                proj_chunk(wq_t, qtiles, QT_t, 0)
                proj_chunk(wk_t, ktiles, KT_t, 0)

                # chunk 0: fused rounds (Kproj one chunk ahead, vproj per jt)
                ch = Chunk(0)
                for jt in range(JT):
                    if jt % JPC == 2 and jt // JPC < SC - 1:
                        proj_chunk(wk_t, ktiles, KT_t, jt // JPC + 1)
                    at_step(ch, jt)
                    vproj(jt)
                    if jt >= LAG:
                        ev_step(ch, jt - LAG)
                for jt in range(JT - LAG, JT):
                    ev_step(ch, jt)
                # boundary: avt copies first (gate next chunk's EV), then PE
                # gets Qproj while the DVE sums chain drains.
                fin = {}
                avt0 = finish_avt(ch)
                proj_chunk(wq_t, qtiles, QT_t, 1)
                inv0 = finish_sums_a(ch)
                finish_sums_b(inv0)
                fin[0] = (avt0, inv0)

                # chunks 1..3: pure at/ev rounds
                for ic in range(1, SC):
                    ch = Chunk(ic, pe_sums=(ic == SC - 1))
                    for jt in range(JT):
                        at_step(ch, jt)
                        if jt >= LAG:
                            ev_step(ch, jt - LAG)
                    for jt in range(JT - LAG, JT):
                        ev_step(ch, jt)
                    avt = finish_avt(ch)
                    if ic == SC - 1:
                        # short tail: inv(3) comes straight off the per-jt PE
                        # sums chain while outproj(ic-1) keeps the PE busy
                        inv = finish_sums_a(ch)
                        pavt, pinv = fin.pop(ic - 1)
                        outproj(ic - 1, pavt, pinv, [0, 1, 2, 3])
                        finish_sums_b(inv)
                        fin[ic] = (avt, inv)
                    else:
                        proj_chunk(wq_t, qtiles, QT_t, ic + 1)
                        inv = finish_sums_a(ch)
                        pavt, pinv = fin.pop(ic - 1)
                        outproj(ic - 1, pavt, pinv, [0, 1, 2, 3])
                        finish_sums_b(inv)
                        fin[ic] = (avt, inv)
                outproj(SC - 1, *fin.pop(SC - 1), [0, 1, 2, 3])

            if reps == 1:
                body()
            else:
                with tc.For_i(0, reps, 1) as i:
                    body(i)

    nc.compile()
    return nc


_CACHE = {}


def _get_program():
    if "nc" not in _CACHE:
        _CACHE["nc"] = build_program(reps=1)
    return _CACHE["nc"]


def _bf(x):
    return np.ascontiguousarray(np.asarray(x, dtype=np.float32)).astype(ml_dtypes.bfloat16)


def _prep_w(wT, nsub):
    # [D_in, D_out] -> pre-tiled SBUF layout [128, nsub * D_out]
    wT = np.asarray(wT, np.float32)
    n = wT.shape[0] // P
    assert n == nsub
    return _bf(wT.reshape(n, P, -1).transpose(1, 0, 2).reshape(P, -1))


def kernel(q, k, v, Wq, Wk, Wv, Wo):
    nc = _get_program()
    # Zero-FLOP host-side layout prep: transpose so the contraction dim (D)
    # lands on SBUF partitions, cast to bf16; one batch per core.
    qT = _bf(np.asarray(q, np.float32).transpose(0, 2, 1))
    kT = _bf(np.asarray(k, np.float32).transpose(0, 2, 1))
    vT = _bf(np.asarray(v, np.float32).transpose(0, 2, 1))
    wql = _prep_w(np.asarray(Wq, np.float32).T, DT)
    wkl = _prep_w(np.asarray(Wk, np.float32).T, DT)
    wvl = _prep_w(np.asarray(Wv, np.float32).T, DT)
    wol = _prep_w(np.asarray(Wo, np.float32).T, RT)

    in_maps = [{"qT": qT[c], "kT": kT[c], "vT": vT[c],
                "wql": wql, "wkl": wkl, "wvl": wvl, "wol": wol}
               for c in range(B)]
    res = bass_utils.run_bass_kernel_spmd(nc, in_maps, core_ids=list(range(B)))
    return np.stack([np.asarray(res.results[c]["out"]).astype(np.float32)
                     for c in range(B)], axis=0)
